# revision 12
# baseline (speedup 1.0000x reference)
"""Trainium2 Bass kernel for nn_Conv_Block (binarized 3x3 conv + BN + binary tanh).

Strategy
--------
Data-parallel over the batch: 32 images -> 4 per core on 8 cores.

Per core:
  * conv3x3(x, sign(w)) computed as 9 shifted matmuls x 2 Cin-chunks of 128,
    accumulated in PSUM.  x is split on the host into bf16 hi + lo parts
    (x = hi + lo to ~2^-16 relative); the binarized weights are exactly
    representable in bf16, so two bf16 matmuls reproduce fp32 precision at
    2x the speed of the native fp32 (4-pass) matmul path.
  * Per-channel partial sums / sums-of-squares accumulated during PSUM
    evacuation (ACT copy w/ accum_out + DVE multiply-reduce).
  * Tiny [128,4] f32 AllReduce across the 8 cores for the BN batch stats.
  * out = Sign(y * a + b) fused into one scalar-engine activation per tile,
    written as bf16 (+-1 exact); host converts to f32.
"""

import numpy as np
import ml_dtypes

import concourse.bass as bass
import concourse.mybir as mybir
import concourse.tile as tile
from concourse import bacc
from concourse import bass_utils

F32 = mybir.dt.float32
BF16 = mybir.dt.bfloat16
EPS = 1e-5


class Cfg:
    def __init__(self, n_cores=8, n_per=4, cin=256, cout=256, h=56, w=56, r=8,
                 use_cc=True, xbufs=2):
        self.use_cc = use_cc
        self.xbufs = xbufs
        self.n_cores = n_cores
        self.n_per = n_per          # images per core
        self.cin = cin
        self.cout = cout
        self.h = h
        self.w = w
        self.r = r                  # output rows per PSUM tile
        self.hp = h + 2             # replicate-padded height
        self.wp = w + 4             # padded width (replicate + zero)
        self.wout = w + 2           # output width
        self.ci_ch = cin // 128
        self.co_ch = cout // 128
        assert h % r == 0
        self.rg = h // r            # row groups per image
        self.t = n_per * self.rg    # stat columns per (core, co_chunk)
        self.npix = h * self.wout
        self.ntot = n_cores * n_per * self.npix  # BN count per channel


def build_program(cfg: Cfg):
    nc = bacc.Bacc(
        "TRN2", target_bir_lowering=False, debug=False, num_devices=cfg.n_cores
    )
    # The ISA constant claims 224KB/partition SBUF, but only 196KB is usable
    # on this part — allocations above that crash the device
    # (NRT_EXEC_UNIT_UNRECOVERABLE). Clamp so overflow fails at compile time.
    nc.sbuf_top = min(nc.sbuf_top, 196 * 1024)

    xhi_d = nc.dram_tensor(
        "xhi", [cfg.n_per, cfg.ci_ch, 128, cfg.hp, cfg.wp], BF16, kind="ExternalInput"
    )
    xlo_d = nc.dram_tensor(
        "xlo", [cfg.n_per, cfg.ci_ch, 128, cfg.hp, cfg.wp], BF16, kind="ExternalInput"
    )
    wts_d = nc.dram_tensor(
        "wts", [128, cfg.ci_ch, 9, cfg.cout], BF16, kind="ExternalInput"
    )
    gb_d = nc.dram_tensor("gb", [128, 2 * cfg.co_ch], F32, kind="ExternalInput")
    out_d = nc.dram_tensor(
        "out",
        [cfg.n_per, cfg.co_ch, 128, cfg.rg, cfg.r, cfg.wout],
        BF16,
        kind="ExternalOutput",
    )
    cc_in_d = nc.dram_tensor("cc_in", [128, 2 * cfg.co_ch], F32)
    cc_out_d = nc.dram_tensor("cc_out", [128, 2 * cfg.co_ch], F32)

    with tile.TileContext(nc) as tc:
        _emit(tc, nc, cfg, xhi_d, xlo_d, wts_d, gb_d, out_d, cc_in_d, cc_out_d)

    nc.compile()
    return nc


def _emit(tc, nc, cfg, xhi_d, xlo_d, wts_d, gb_d, out_d, cc_in_d, cc_out_d):
    CI, CO, RG, R, WOUT, T = cfg.ci_ch, cfg.co_ch, cfg.rg, cfg.r, cfg.wout, cfg.t

    with (
        tc.tile_pool(name="singles", bufs=1) as singles,
        tc.tile_pool(name="xpool", bufs=cfg.xbufs) as xpool,
        tc.tile_pool(name="psum", bufs=4, space="PSUM") as psum,
        tc.tile_pool(name="evac", bufs=2) as evac,
        tc.tile_pool(name="ypool", bufs=1) as ypool,
        tc.tile_pool(name="outp", bufs=4) as outp,
    ):
        w_sb = singles.tile([128, CI, 9, cfg.cout], BF16)
        nc.sync.dma_start(out=w_sb, in_=wts_d.ap())
        gb_sb = singles.tile([128, 2 * CO], F32)
        nc.sync.dma_start(out=gb_sb, in_=gb_d.ap())

        sums = singles.tile([128, CO, T], F32)
        sumsq = singles.tile([128, CO, T], F32)

        y_tiles = {}
        for i in range(cfg.n_per):
            xhi_sb = xpool.tile([128, CI, cfg.hp, cfg.wp], BF16, tag="xhi", name=f"xhi_{i}")
            xlo_sb = xpool.tile([128, CI, cfg.hp, cfg.wp], BF16, tag="xlo", name=f"xlo_{i}")
            for c in range(CI):
                nc.sync.dma_start(out=xhi_sb[:, c], in_=xhi_d.ap()[i, c])
                nc.sync.dma_start(out=xlo_sb[:, c], in_=xlo_d.ap()[i, c])

            for k in range(CO):
                y_sb = ypool.tile(
                    [128, RG, R, WOUT], F32, tag=f"y_{i}_{k}", name=f"y_{i}_{k}"
                )
                y_tiles[(i, k)] = y_sb
                for g in range(RG):
                    ps = psum.tile([128, R, WOUT], F32, tag="ps", name=f"ps_{i}_{k}_{g}")
                    n_mm = 9 * CI * 2
                    mm = 0
                    for tap in range(9):
                        kh, kw = divmod(tap, 3)
                        for c in range(CI):
                            lhsT = w_sb[:, c, tap, k * 128 : (k + 1) * 128]
                            rhs_hi = xhi_sb[
                                :, c, g * R + kh : g * R + kh + R, kw : kw + WOUT
                            ]
                            rhs_lo = xlo_sb[
                                :, c, g * R + kh : g * R + kh + R, kw : kw + WOUT
                            ]
                            nc.tensor.matmul(
                                ps, lhsT, rhs_hi, start=(mm == 0), stop=(mm == n_mm - 1)
                            )
                            mm += 1
                            nc.tensor.matmul(
                                ps, lhsT, rhs_lo, start=False, stop=(mm == n_mm - 1)
                            )
                            mm += 1
                    t = i * RG + g
                    # evacuate PSUM -> y (f32) + per-partition sum
                    nc.scalar.activation(
                        y_sb[:, g],
                        ps,
                        mybir.ActivationFunctionType.Copy,
                        accum_out=sums[:, k, t : t + 1],
                    )
                    # sum of squares via a second ACT pass
                    # (vector.tensor_tensor_reduce crashes HW through this stack)
                    scr = evac.tile([128, R, WOUT], F32, tag="scr", name=f"scr_{i}_{k}_{g}")
                    nc.scalar.activation(
                        scr,
                        ps,
                        mybir.ActivationFunctionType.Square,
                        accum_out=sumsq[:, k, t : t + 1],
                    )

        # ---- BN statistics: local reduce, cross-core all-reduce ----
        cc_in = singles.tile([128, 2 * CO], F32)
        nc.vector.tensor_reduce(
            out=cc_in[:, 0:CO], in_=sums, axis=mybir.AxisListType.X,
            op=mybir.AluOpType.add,
        )
        nc.vector.tensor_reduce(
            out=cc_in[:, CO : 2 * CO], in_=sumsq, axis=mybir.AxisListType.X,
            op=mybir.AluOpType.add,
        )
        tot = singles.tile([128, 2 * CO], F32)
        if cfg.use_cc:
            nc.sync.dma_start(out=cc_in_d.ap(), in_=cc_in)
            nc.gpsimd.collective_compute(
                "AllReduce",
                mybir.AluOpType.add,
                replica_groups=[list(range(cfg.n_cores))],
                ins=[cc_in_d.ap()],
                outs=[cc_out_d.ap()],
            )
            nc.sync.dma_start(out=tot, in_=cc_out_d.ap())
        else:
            # per-core stats (debug / two-kernel fallback): scale count to
            # n_cores*count so 1/ntot still gives this core's mean.
            nc.scalar.mul(tot, cc_in, float(cfg.n_cores))

        # ---- per-channel scale/shift: a = gamma*rsqrt(var+eps), b = beta - mean*a
        eps_t = singles.tile([128, 1], F32)
        nc.vector.memset(eps_t, EPS)
        meanv = singles.tile([128, 2 * CO], F32)  # [mean | E(x^2)]
        nc.scalar.mul(meanv, tot, 1.0 / cfg.ntot)
        msq = singles.tile([128, CO], F32)
        nc.vector.tensor_mul(msq, meanv[:, 0:CO], meanv[:, 0:CO])
        var_t = singles.tile([128, CO], F32)
        nc.vector.tensor_sub(var_t, meanv[:, CO : 2 * CO], msq)
        std_t = singles.tile([128, CO], F32)
        nc.scalar.activation(
            std_t, var_t, mybir.ActivationFunctionType.Sqrt, bias=eps_t
        )
        rstd = singles.tile([128, CO], F32)
        nc.vector.reciprocal(rstd, std_t)
        a_t = singles.tile([128, CO], F32)
        nc.vector.tensor_mul(a_t, gb_sb[:, 0:CO], rstd)
        ma = singles.tile([128, CO], F32)
        nc.vector.tensor_mul(ma, meanv[:, 0:CO], a_t)
        b_t = singles.tile([128, CO], F32)
        nc.vector.tensor_sub(b_t, gb_sb[:, CO : 2 * CO], ma)

        # ---- binarize: out = Sign(y*a + b), per row-group to keep SBUF low ----
        for i in range(cfg.n_per):
            for k in range(CO):
                for g in range(RG):
                    o_sb = outp.tile(
                        [128, R, WOUT], BF16, tag="o", name=f"o_{i}_{k}_{g}"
                    )
                    nc.scalar.activation(
                        o_sb,
                        y_tiles[(i, k)][:, g],
                        mybir.ActivationFunctionType.Sign,
                        bias=b_t[:, k : k + 1],
                        scale=a_t[:, k : k + 1],
                    )
                    nc.sync.dma_start(out=out_d.ap()[i, k, :, g], in_=o_sb)


# ---------------------------------------------------------------------------
# host side
# ---------------------------------------------------------------------------

def prep_inputs(cfg: Cfg, x, weight, gamma, beta):
    """Full inputs -> list of per-core in_maps."""
    x = np.asarray(x, dtype=np.float32)
    weight = np.asarray(weight, dtype=np.float32)
    gamma = np.asarray(gamma, dtype=np.float32)
    beta = np.asarray(beta, dtype=np.float32)

    xp = np.pad(x, ((0, 0), (0, 0), (1, 1), (1, 1)), mode="edge")
    xp = np.pad(xp, ((0, 0), (0, 0), (0, 0), (1, 1)), mode="constant")
    hi = xp.astype(ml_dtypes.bfloat16)
    lo = (xp - hi.astype(np.float32)).astype(ml_dtypes.bfloat16)
    n = cfg.n_cores * cfg.n_per
    hi = hi.reshape(n, cfg.ci_ch, 128, cfg.hp, cfg.wp)
    lo = lo.reshape(n, cfg.ci_ch, 128, cfg.hp, cfg.wp)

    sw = np.where(weight >= 0, np.float32(1.0), np.float32(-1.0))
    # [co, ci_ch, ci, tap] -> [ci, ci_ch, tap, co]
    wts = sw.reshape(cfg.cout, cfg.ci_ch, 128, 9).transpose(2, 1, 3, 0)
    wts = np.ascontiguousarray(wts).astype(ml_dtypes.bfloat16)

    g_cols = gamma.reshape(cfg.co_ch, 128).T  # [128, co_ch]
    b_cols = beta.reshape(cfg.co_ch, 128).T
    gb = np.ascontiguousarray(np.concatenate([g_cols, b_cols], axis=1), dtype=np.float32)

    in_maps = []
    for c in range(cfg.n_cores):
        s = slice(c * cfg.n_per, (c + 1) * cfg.n_per)
        in_maps.append(
            {"xhi": np.ascontiguousarray(hi[s]), "xlo": np.ascontiguousarray(lo[s]),
             "wts": wts, "gb": gb}
        )
    return in_maps


def unshard_output(cfg: Cfg, per_core_outs):
    out = np.concatenate(per_core_outs, axis=0)  # [N, co_ch, 128, rg, r, wout]
    n = cfg.n_cores * cfg.n_per
    return out.astype(np.float32).reshape(n, cfg.cout, cfg.h, cfg.wout)


_CACHE = {}


def _get_program(cfg: Cfg):
    key = (cfg.n_cores, cfg.n_per, cfg.cin, cfg.cout, cfg.h, cfg.w, cfg.r,
           cfg.use_cc, cfg.xbufs)
    if key not in _CACHE:
        _CACHE[key] = build_program(cfg)
    return _CACHE[key]


def run(x, weight, gamma, beta, trace=False, cfg: Cfg | None = None):
    cfg = cfg or Cfg()
    nc = _get_program(cfg)
    in_maps = prep_inputs(cfg, x, weight, gamma, beta)
    res = bass_utils.run_bass_kernel_spmd(
        nc, in_maps, core_ids=list(range(cfg.n_cores)), trace=trace
    )
    out = unshard_output(cfg, [r["out"] for r in res.results])
    return out, res


def kernel(x, weight, gamma, beta):
    out, _ = run(x, weight, gamma, beta)
    return out


# revision 23
# speedup vs baseline: 1.4771x; 1.4771x over previous
"""Trainium2 Bass kernel for nn_Conv_Block (binarized 3x3 conv + BN + binary tanh).

Strategy
--------
Data-parallel over the batch: 32 images -> 4 per core on 8 cores.

Per core:
  * conv3x3(x, sign(w)) computed as 9 shifted matmuls x 2 Cin-chunks of 128,
    accumulated in PSUM.  x is split on the host into bf16 hi + lo parts
    (x = hi + lo to ~2^-16 relative); the binarized weights are exactly
    representable in bf16, so two bf16 matmuls reproduce fp32 precision at
    2x the speed of the native fp32 (4-pass) matmul path.
  * Loops run channel-chunk-major: all of chunk k's conv finishes, then its
    BN stats AllReduce (tiny [128,2] f32) and its Sign pass overlap the next
    chunk's conv on the tensor engine.
  * Per-channel sums / sums-of-squares accumulated during PSUM evacuation
    via scalar-engine activations with accum_out (the DVE tensor_tensor_reduce
    path crashes this hardware stack).
  * out = Sign(y * a + b) fused into one scalar-engine activation per
    row-group, written as fp8e4m3 (+-1 exact); host converts to f32.

Environment constraints baked in (see memory/trn2-axon-env-pitfalls):
usable SBUF is 196KB/partition (not the 224KB the ISA reports) — exceeding it
crashes the device with NRT_EXEC_UNIT_UNRECOVERABLE.
"""

import numpy as np
import ml_dtypes

import concourse.bass as bass
import concourse.mybir as mybir
import concourse.tile as tile
from concourse import bacc
from concourse import bass_utils

F32 = mybir.dt.float32
BF16 = mybir.dt.bfloat16
FP8 = mybir.dt.float8e4  # +-1 is exact; halves the output DMA vs bf16
EPS = 1e-5


class Cfg:
    def __init__(self, n_cores=8, n_per=4, cin=256, cout=256, h=56, w=56, r=8,
                 use_cc=True, xbufs=2, reps=1, conv_only=False):
        self.use_cc = use_cc
        self.conv_only = conv_only  # timing/debug: skip stats + sign phases
        self.xbufs = xbufs
        self.reps = reps            # timing-only: repeat the conv phase
        self.n_cores = n_cores
        self.n_per = n_per          # images per core
        self.cin = cin
        self.cout = cout
        self.h = h
        self.w = w
        self.r = r                  # output rows per PSUM tile
        self.hp = h + 2             # replicate-padded height
        self.wp = w + 4             # padded width (replicate + zero)
        self.wout = w + 2           # output width
        self.ci_ch = cin // 128
        self.co_ch = cout // 128
        assert h % r == 0
        self.rg = h // r            # row groups per image
        self.t = n_per * self.rg    # stat columns per (core, co_chunk)
        self.npix = h * self.wout
        self.ntot = n_cores * n_per * self.npix  # BN count per channel


def build_program(cfg: Cfg):
    nc = bacc.Bacc(
        "TRN2", target_bir_lowering=False, debug=False, num_devices=cfg.n_cores
    )
    # The ISA constant claims 224KB/partition SBUF, but only 196KB is usable
    # on this part — allocations above that crash the device
    # (NRT_EXEC_UNIT_UNRECOVERABLE). Clamp so overflow fails at compile time.
    nc.sbuf_top = min(nc.sbuf_top, 196 * 1024)

    xhi_d = nc.dram_tensor(
        "xhi", [cfg.n_per, cfg.ci_ch, 128, cfg.hp, cfg.wp], BF16, kind="ExternalInput"
    )
    xlo_d = nc.dram_tensor(
        "xlo", [cfg.n_per, cfg.ci_ch, 128, cfg.hp, cfg.wp], BF16, kind="ExternalInput"
    )
    wts_d = nc.dram_tensor(
        "wts", [128, cfg.ci_ch, 9, cfg.cout], BF16, kind="ExternalInput"
    )
    gb_d = nc.dram_tensor("gb", [128, 2 * cfg.co_ch], F32, kind="ExternalInput")
    out_d = nc.dram_tensor(
        "out",
        [cfg.n_per, cfg.co_ch, 128, cfg.rg, cfg.r, cfg.wout],
        FP8,
        kind="ExternalOutput",
    )
    cc_in_d = [
        nc.dram_tensor(f"cc_in_{k}", [128, 2], F32) for k in range(cfg.co_ch)
    ]
    cc_out_d = [
        nc.dram_tensor(f"cc_out_{k}", [128, 2], F32) for k in range(cfg.co_ch)
    ]

    with tile.TileContext(nc) as tc:
        _emit(tc, nc, cfg, xhi_d, xlo_d, wts_d, gb_d, out_d, cc_in_d, cc_out_d)

    nc.compile()
    return nc


def _emit(tc, nc, cfg, xhi_d, xlo_d, wts_d, gb_d, out_d, cc_in_d, cc_out_d):
    CI, CO, RG, R, WOUT, T = cfg.ci_ch, cfg.co_ch, cfg.rg, cfg.r, cfg.wout, cfg.t

    with (
        tc.tile_pool(name="singles", bufs=1) as singles,
        tc.tile_pool(name="xpool", bufs=cfg.xbufs) as xpool,
        tc.tile_pool(name="psum", bufs=4, space="PSUM") as psum,
        tc.tile_pool(name="evac", bufs=2) as evac,
        tc.tile_pool(name="ypool", bufs=1) as ypool,
        tc.tile_pool(name="outp", bufs=4) as outp,
    ):
        w_sb = singles.tile([128, CI, 9, cfg.cout], BF16)
        nc.sync.dma_start(out=w_sb, in_=wts_d.ap())
        gb_sb = singles.tile([128, 2 * CO], F32)
        nc.sync.dma_start(out=gb_sb, in_=gb_d.ap())
        eps_t = singles.tile([128, 1], F32)
        nc.vector.memset(eps_t, EPS)

        for rep in range(cfg.reps):
          do_post = (rep == cfg.reps - 1) and not cfg.conv_only
          for k in range(CO):
            sums_k = singles.tile([128, T], F32, name=f"sums_{rep}_{k}")
            sumsq_k = singles.tile([128, T], F32, name=f"sumsq_{rep}_{k}")
            y_tiles = []
            for i in range(cfg.n_per):
                xhi_sb = xpool.tile([128, CI, cfg.hp, cfg.wp], BF16, tag="xhi",
                                    name=f"xhi_{rep}_{k}_{i}")
                xlo_sb = xpool.tile([128, CI, cfg.hp, cfg.wp], BF16, tag="xlo",
                                    name=f"xlo_{rep}_{k}_{i}")
                for c in range(CI):
                    nc.sync.dma_start(out=xhi_sb[:, c], in_=xhi_d.ap()[i, c])
                    nc.sync.dma_start(out=xlo_sb[:, c], in_=xlo_d.ap()[i, c])

                y_sb = ypool.tile(
                    [128, RG, R, WOUT], F32, tag=f"y_{k}_{i}", name=f"y_{rep}_{k}_{i}"
                )
                y_tiles.append(y_sb)
                for g in range(RG):
                    ps = psum.tile([128, R, WOUT], F32, tag="ps",
                                   name=f"ps_{rep}_{k}_{i}_{g}")
                    n_mm = 9 * CI * 2
                    mm = 0
                    # source-major order: the first matmul group only needs
                    # the first DMA (hi chunk 0) to have landed.
                    srcs = [(xhi_sb, c) for c in range(CI)] + \
                           [(xlo_sb, c) for c in range(CI)]
                    for xsb, c in srcs:
                        for tap in range(9):
                            kh, kw = divmod(tap, 3)
                            lhsT = w_sb[:, c, tap, k * 128 : (k + 1) * 128]
                            rhs = xsb[
                                :, c, g * R + kh : g * R + kh + R, kw : kw + WOUT
                            ]
                            nc.tensor.matmul(
                                ps, lhsT, rhs, start=(mm == 0), stop=(mm == n_mm - 1)
                            )
                            mm += 1
                    t = i * RG + g
                    # evacuate PSUM -> y (f32) + per-partition sum
                    nc.scalar.activation(
                        y_sb[:, g],
                        ps,
                        mybir.ActivationFunctionType.Copy,
                        accum_out=sums_k[:, t : t + 1],
                    )
                    # sum of squares via a second ACT pass
                    # (vector.tensor_tensor_reduce crashes HW through this stack)
                    scr = evac.tile([128, R, WOUT], F32, tag="scr",
                                    name=f"scr_{rep}_{k}_{i}_{g}")
                    nc.scalar.activation(
                        scr,
                        ps,
                        mybir.ActivationFunctionType.Square,
                        accum_out=sumsq_k[:, t : t + 1],
                    )

            if not do_post:
                continue

            # ---- BN statistics for chunk k: local reduce + tiny AllReduce ----
            cc_sb = singles.tile([128, 2], F32, name=f"cc_sb_{k}")
            nc.vector.tensor_reduce(
                out=cc_sb[:, 0:1], in_=sums_k, axis=mybir.AxisListType.X,
                op=mybir.AluOpType.add,
            )
            nc.vector.tensor_reduce(
                out=cc_sb[:, 1:2], in_=sumsq_k, axis=mybir.AxisListType.X,
                op=mybir.AluOpType.add,
            )
            tot = singles.tile([128, 2], F32, name=f"tot_{k}")
            if cfg.use_cc:
                nc.sync.dma_start(out=cc_in_d[k].ap(), in_=cc_sb)
                nc.gpsimd.collective_compute(
                    "AllReduce",
                    mybir.AluOpType.add,
                    replica_groups=[list(range(cfg.n_cores))],
                    ins=[cc_in_d[k].ap()],
                    outs=[cc_out_d[k].ap()],
                )
                nc.sync.dma_start(out=tot, in_=cc_out_d[k].ap())
            else:
                # per-core stats (debug): emulate the all-reduce sum scale
                nc.scalar.mul(tot, cc_sb, float(cfg.n_cores))

            # ---- a = gamma*rsqrt(var+eps), b = beta - mean*a  (chunk k) ----
            mv = singles.tile([128, 2], F32, name=f"mv_{k}")  # [mean, E(y^2)]
            nc.scalar.mul(mv, tot, 1.0 / cfg.ntot)
            msq = singles.tile([128, 1], F32, name=f"msq_{k}")
            nc.vector.tensor_mul(msq, mv[:, 0:1], mv[:, 0:1])
            var_t = singles.tile([128, 1], F32, name=f"var_{k}")
            nc.vector.tensor_sub(var_t, mv[:, 1:2], msq)
            std_t = singles.tile([128, 1], F32, name=f"std_{k}")
            nc.scalar.activation(
                std_t, var_t, mybir.ActivationFunctionType.Sqrt, bias=eps_t
            )
            rstd = singles.tile([128, 1], F32, name=f"rstd_{k}")
            nc.vector.reciprocal(rstd, std_t)
            a_t = singles.tile([128, 1], F32, name=f"a_{k}")
            nc.vector.tensor_mul(a_t, gb_sb[:, k : k + 1], rstd)
            ma = singles.tile([128, 1], F32, name=f"ma_{k}")
            nc.vector.tensor_mul(ma, mv[:, 0:1], a_t)
            b_t = singles.tile([128, 1], F32, name=f"b_{k}")
            nc.vector.tensor_sub(b_t, gb_sb[:, CO + k : CO + k + 1], ma)

            # ---- binarize chunk k: out = Sign(y*a + b), per row-group ----
            # (overlaps the next chunk's conv on PE)
            for i in range(cfg.n_per):
                for g in range(RG):
                    o_sb = outp.tile(
                        [128, R, WOUT], FP8, tag="o", name=f"o_{k}_{i}_{g}"
                    )
                    nc.scalar.activation(
                        o_sb,
                        y_tiles[i][:, g],
                        mybir.ActivationFunctionType.Sign,
                        bias=b_t,
                        scale=a_t,
                    )
                    nc.sync.dma_start(out=out_d.ap()[i, k, :, g], in_=o_sb)

        if cfg.conv_only:
            o_sb = outp.tile([128, R, WOUT], FP8, tag="o", name="o_flush")
            nc.scalar.activation(
                o_sb, y_tiles[0][:, 0], mybir.ActivationFunctionType.Sign
            )
            nc.sync.dma_start(out=out_d.ap()[0, 0, :, 0], in_=o_sb)


# ---------------------------------------------------------------------------
# host side
# ---------------------------------------------------------------------------

def prep_inputs(cfg: Cfg, x, weight, gamma, beta):
    """Full inputs -> list of per-core in_maps."""
    x = np.asarray(x, dtype=np.float32)
    weight = np.asarray(weight, dtype=np.float32)
    gamma = np.asarray(gamma, dtype=np.float32)
    beta = np.asarray(beta, dtype=np.float32)

    xp = np.pad(x, ((0, 0), (0, 0), (1, 1), (1, 1)), mode="edge")
    xp = np.pad(xp, ((0, 0), (0, 0), (0, 0), (1, 1)), mode="constant")
    hi = xp.astype(ml_dtypes.bfloat16)
    lo = (xp - hi.astype(np.float32)).astype(ml_dtypes.bfloat16)
    n = cfg.n_cores * cfg.n_per
    hi = hi.reshape(n, cfg.ci_ch, 128, cfg.hp, cfg.wp)
    lo = lo.reshape(n, cfg.ci_ch, 128, cfg.hp, cfg.wp)

    sw = np.where(weight >= 0, np.float32(1.0), np.float32(-1.0))
    # [co, ci_ch, ci, tap] -> [ci, ci_ch, tap, co]
    wts = sw.reshape(cfg.cout, cfg.ci_ch, 128, 9).transpose(2, 1, 3, 0)
    wts = np.ascontiguousarray(wts).astype(ml_dtypes.bfloat16)

    g_cols = gamma.reshape(cfg.co_ch, 128).T  # [128, co_ch]
    b_cols = beta.reshape(cfg.co_ch, 128).T
    gb = np.ascontiguousarray(np.concatenate([g_cols, b_cols], axis=1), dtype=np.float32)

    in_maps = []
    for c in range(cfg.n_cores):
        s = slice(c * cfg.n_per, (c + 1) * cfg.n_per)
        in_maps.append(
            {"xhi": np.ascontiguousarray(hi[s]), "xlo": np.ascontiguousarray(lo[s]),
             "wts": wts, "gb": gb}
        )
    return in_maps


def unshard_output(cfg: Cfg, per_core_outs):
    out = np.concatenate(per_core_outs, axis=0)  # [N, co_ch, 128, rg, r, wout]
    n = cfg.n_cores * cfg.n_per
    return out.astype(np.float32).reshape(n, cfg.cout, cfg.h, cfg.wout)


_CACHE = {}


def _get_program(cfg: Cfg):
    key = (cfg.n_cores, cfg.n_per, cfg.cin, cfg.cout, cfg.h, cfg.w, cfg.r,
           cfg.use_cc, cfg.xbufs, cfg.reps, cfg.conv_only)
    if key not in _CACHE:
        _CACHE[key] = build_program(cfg)
    return _CACHE[key]


def run(x, weight, gamma, beta, trace=False, cfg: Cfg | None = None):
    cfg = cfg or Cfg()
    nc = _get_program(cfg)
    in_maps = prep_inputs(cfg, x, weight, gamma, beta)
    res = bass_utils.run_bass_kernel_spmd(
        nc, in_maps, core_ids=list(range(cfg.n_cores)), trace=trace
    )
    out = unshard_output(cfg, [r["out"] for r in res.results])
    return out, res


def kernel(x, weight, gamma, beta):
    out, _ = run(x, weight, gamma, beta)
    return out
